# revision 31
# baseline (speedup 1.0000x reference)
"""Adaptive pooling layer on 8 Trainium2 NeuronCores.

Data-parallel over batch: B=16 -> 2 batches per core, all parameters
replicated, no collectives. Per batch the kernel computes

  m    = mean(node)                          (DVE reduce of node^T)
  c    = relu-MLP(m)                [D, H*K]
  S    = (node @ (c * 1/|cent|)) * 1/|node|  [n, H*K]   (cosine sims)
  Cn   = S / (sum_k S + 1e-10)               (normalize over centroids)
  C^T  = sum_h w_h * Cn[:, h, :] + conv_b    [n, K]
  new_node = (C @ node) @ lin_w^T + lin_b    [K, DOUT]
  T    = C @ adj   (f32r, streams the 64MB adj = the memory roofline)
  new_adj  = relu(T @ C^T)                   [K, K]

Numerical care: the K-normalizer sum_k S cancels to ~1e-6 for a few
(h, n) while |S|~1e-2, amplifying rounding ~1000x into the dominant
|C|~800 entries. The dot products are therefore computed with 16
row-masked matmuls (zero rows add exact zeros), giving seq-8 product
accumulation like XLA's blocked f32 dot, combined sequentially in PSUM
per 64-row half; sqrt uses a Newton-refined ACT LUT. This lands within
~5e-3 of the f32 jax reference (vs ~2.5e-2 for a naive f32 chain).

Self-contained: only numpy + concourse imports, shapes hardcoded.
"""

import math
from contextlib import ExitStack

import numpy as np

import concourse.bass as bass
import concourse.mybir as mybir
import concourse.tile as tile
from concourse import bacc
from concourse.bass_utils import run_bass_kernel_spmd

F32 = mybir.dt.float32
F32R = mybir.dt.float32r
AF = mybir.ActivationFunctionType
ALU = mybir.AluOpType

# Problem shapes (hardcoded per harness contract)
B, N, D, DOUT = 16, 4000, 128, 128
H, K = 4, 32
HK = H * K            # 128
H1, H2 = 16, 64
NCORES = 8
BPC = B // NCORES     # batches per core
P = 128
OUTW = DOUT + K       # packed output row width (new_node | new_adj)

MCH = 512             # adj column-chunk width (one PSUM bank, f32)


def _tile_rows(t, n):
    return min(P, n - t * P)


def build_program(nc: bass.Bass, n_nodes: int = N, bpc: int = BPC,
                  adj_bufs: int = 30):
    NT = (n_nodes + P - 1) // P
    LAST = n_nodes - (NT - 1) * P

    node_e = nc.declare_dram_parameter("nodes", [bpc, n_nodes, D], F32, isOutput=False)
    adj_e = nc.declare_dram_parameter("adj", [bpc, n_nodes, n_nodes], F32R, isOutput=False)
    w1_e = nc.declare_dram_parameter("w1", [H1, 1], F32, isOutput=False)
    b1_e = nc.declare_dram_parameter("b1", [H1], F32, isOutput=False)
    W2_e = nc.declare_dram_parameter("W2", [H2, H1], F32, isOutput=False)
    b2_e = nc.declare_dram_parameter("b2", [H2], F32, isOutput=False)
    W3_e = nc.declare_dram_parameter("W3", [HK, H2], F32, isOutput=False)
    b3_e = nc.declare_dram_parameter("b3", [HK], F32, isOutput=False)
    cw_e = nc.declare_dram_parameter("conv_w", [1, H, 1, 1], F32, isOutput=False)
    cb_e = nc.declare_dram_parameter("conv_b", [1], F32, isOutput=False)
    lw_e = nc.declare_dram_parameter("lin_w", [DOUT, D], F32, isOutput=False)
    lb_e = nc.declare_dram_parameter("lin_b", [DOUT], F32, isOutput=False)
    out_e = nc.declare_dram_parameter("out", [bpc, K, OUTW], F32, isOutput=True)

    ident_d = nc.inline_tensor(np.eye(P, dtype=np.float32), name="ident_const")
    # mask16[:, j] selects d-rows [8j, 8j+8): multiplying cs by a mask makes
    # a contract-64 matmul numerically a seq-8 partial dot (zero rows add
    # exact zeros), reproducing XLA's blocked f32 accumulation.
    m16 = np.zeros((P, 16), dtype=np.float32)
    for j in range(16):
        m16[8 * j : 8 * j + 8, j] = 1.0
    mask_d = nc.inline_tensor(m16, name="mask16_const")

    with ExitStack() as ctx:
        tc = ctx.enter_context(tile.TileContext(nc))
        const = ctx.enter_context(tc.tile_pool(name="const", bufs=1))
        pnode = ctx.enter_context(tc.tile_pool(name="pnode", bufs=2))
        pndt = ctx.enter_context(tc.tile_pool(name="pndt", bufs=1))
        ps_all = ctx.enter_context(tc.tile_pool(name="ps_all", bufs=1))
        pct = ctx.enter_context(tc.tile_pool(name="pct", bufs=2))
        psmall = ctx.enter_context(tc.tile_pool(name="psmall", bufs=2))
        padj = ctx.enter_context(tc.tile_pool(name="padj", bufs=adj_bufs))
        ptsb = ctx.enter_context(tc.tile_pool(name="ptsb", bufs=2))
        ppd = ctx.enter_context(tc.tile_pool(name="ppd", bufs=1, space="PSUM"))
        ppa = ctx.enter_context(tc.tile_pool(name="ppa", bufs=2, space="PSUM"))
        ppc = ctx.enter_context(tc.tile_pool(name="ppc", bufs=2, space="PSUM"))
        ppt = ctx.enter_context(tc.tile_pool(name="ppt", bufs=1, space="PSUM"))
        ppn = ctx.enter_context(tc.tile_pool(name="ppn", bufs=1, space="PSUM"))

        # ---- constants / replicated parameters ----
        ident = const.tile([P, P], F32)
        nc.sync.dma_start(out=ident, in_=ident_d[:, :])
        ones_row = const.tile([1, P], F32)
        nc.vector.memset(ones_row, 1.0)
        ones_col = const.tile([P, 1], F32)
        nc.vector.memset(ones_col, 1.0)

        w1bc = const.tile([P, H1], F32)
        nc.sync.dma_start(out=w1bc, in_=w1_e[:, 0].unsqueeze(0).partition_broadcast(P))
        b1bc = const.tile([P, H1], F32)
        nc.sync.dma_start(out=b1bc, in_=b1_e[:].unsqueeze(0).partition_broadcast(P))
        cwbc = const.tile([P, H], F32)
        nc.sync.dma_start(out=cwbc, in_=cw_e[0, :, 0, 0].unsqueeze(0).partition_broadcast(P))
        cbbc = const.tile([P, 1], F32)
        nc.sync.dma_start(out=cbbc, in_=cb_e[:].unsqueeze(0).partition_broadcast(P))
        b2row = const.tile([1, H2], F32)
        nc.sync.dma_start(out=b2row, in_=b2_e[:].unsqueeze(0))
        b3row = const.tile([1, HK], F32)
        nc.sync.dma_start(out=b3row, in_=b3_e[:].unsqueeze(0))
        lbrow = const.tile([1, DOUT], F32)
        nc.sync.dma_start(out=lbrow, in_=lb_e[:].unsqueeze(0))

        mask16 = const.tile([P, 16], F32)
        nc.sync.dma_start(out=mask16, in_=mask_d[:, :])

        W2sb = const.tile([H2, H1], F32)
        nc.sync.dma_start(out=W2sb, in_=W2_e[:, :])
        W3sb = const.tile([HK, H2], F32)
        nc.sync.dma_start(out=W3sb, in_=W3_e[:, :])
        lwsb = const.tile([DOUT, D], F32)
        nc.sync.dma_start(out=lwsb, in_=lw_e[:, :])

        W2T = const.tile([H1, H2], F32)
        W3T = const.tile([H2, HK], F32)
        lwT = const.tile([D, DOUT], F32)
        for src, dst in ((W2sb, W2T), (W3sb, W3T), (lwsb, lwT)):
            p_in, f_in = src.shape
            tp = ppa.tile([P, P], F32, tag="ppa")
            nc.tensor.transpose(tp[:f_in, :p_in], src, ident[:p_in, :p_in])
            nc.scalar.copy(out=dst, in_=tp[:f_in, :p_in])

        # ---- per-batch state ----
        ctr_tiles = [None] * bpc    # C^T rounded to f32r for the big matmul
        ct_tiles = [None] * bpc     # C^T  [128, NT, K]
        node_tiles = [None] * bpc   # node [128, NT, D]
        stage_tiles = [None] * bpc  # packed output rows [K, OUTW]

        def nr_sqrt_recip(out_r, x, tmp_a, tmp_b):
            """out_r = 1/max(sqrt(x), 1e-30), Newton-refined LUT sqrt.

            x, out_r, tmp_a, tmp_b: same-shape SBUF APs; x preserved.
            """
            nc.scalar.activation(out=tmp_a, in_=x, func=AF.Sqrt)          # y0
            nc.vector.tensor_scalar_max(out=tmp_a, in0=tmp_a, scalar1=1e-30)
            nc.vector.reciprocal(out=tmp_b, in_=tmp_a)                    # 1/y0
            nc.vector.tensor_mul(tmp_b, x, tmp_b)                         # x/y0
            nc.vector.tensor_add(tmp_a, tmp_a, tmp_b)                     # y0 + x/y0
            nc.vector.tensor_scalar_mul(out=tmp_a, in0=tmp_a, scalar1=0.5)  # y1
            nc.vector.tensor_scalar_max(out=tmp_a, in0=tmp_a, scalar1=1e-30)
            nc.vector.reciprocal(out=out_r, in_=tmp_a)

        def sim_phase(b):
            node_sb = pnode.tile([P, NT, D], F32, tag="node")
            node_tiles[b] = node_sb
            if LAST < P:
                nc.sync.dma_start(
                    out=node_sb[:, : NT - 1, :],
                    in_=node_e[b, : (NT - 1) * P, :].rearrange("(t p) d -> p t d", p=P),
                )
                nc.sync.dma_start(
                    out=node_sb[:LAST, NT - 1, :],
                    in_=node_e[b, (NT - 1) * P :, :],
                )
            else:
                nc.sync.dma_start(
                    out=node_sb,
                    in_=node_e[b, :, :].rearrange("(t p) d -> p t d", p=P),
                )

            ndT = pndt.tile([P, NT, P], F32, tag="ndt")
            nn2 = psmall.tile([P, NT], F32, tag="nn2")
            nn_r = psmall.tile([P, NT], F32, tag="nn_r")
            sqs = psmall.tile([P, D], F32, tag="sqs")
            tmp_a = psmall.tile([P, NT], F32, tag="tmp_a")
            tmp_b = psmall.tile([P, NT], F32, tag="tmp_b")
            nc.vector.memset(nn2, 0.0)
            for t in range(NT):
                pt = _tile_rows(t, n_nodes)
                tp = ppa.tile([P, P], F32, tag="ppa")
                nc.tensor.transpose(tp[:D, :pt], node_sb[:pt, t, :], ident[:pt, :pt])
                nc.scalar.copy(out=ndT[:, t, :pt], in_=tp[:D, :pt])
                nc.scalar.activation(
                    out=sqs[:pt, :], in_=node_sb[:pt, t, :], func=AF.Square,
                    accum_out=nn2[:pt, t : t + 1],
                )
            if LAST < P:
                nc.vector.memset(ndT[:, NT - 1, LAST:], 0.0)

            # 1/|node_n| (exactly-zero norms -> huge-but-finite scale; the
            # matching dots are exactly 0, mirroring the reference clamp)
            nr_sqrt_recip(nn_r, nn2, tmp_a, tmp_b)

            # mean over nodes -> [D, 1] column
            m_col = psmall.tile([P, 1], F32, tag="m_col")
            nc.vector.reduce_sum(
                out=m_col, in_=ndT.rearrange("p t n -> p (t n)"),
                axis=mybir.AxisListType.X,
            )

            # --- MLP: c1 = relu(m*w1 + b1) via DVE ---
            c1 = psmall.tile([P, H1], F32, tag="c1")
            nc.vector.tensor_scalar(
                out=c1, in0=w1bc, scalar1=m_col, scalar2=1.0 / float(n_nodes),
                op0=ALU.mult, op1=ALU.mult,
            )
            nc.vector.tensor_add(c1, c1, b1bc)
            nc.scalar.activation(out=c1, in_=c1, func=AF.Relu)

            c1T = psmall.tile([H1, P], F32, tag="c1T")
            tp = ppa.tile([P, P], F32, tag="ppa")
            nc.tensor.transpose(tp[:H1, :P], c1, ident)
            nc.scalar.copy(out=c1T, in_=tp[:H1, :P])

            c2 = psmall.tile([P, H2], F32, tag="c2")
            mp = ppa.tile([P, P], F32, tag="ppa")
            nc.tensor.matmul(mp[:, :H2], c1T, W2T, start=True, stop=False,
                             skip_group_check=True)
            nc.tensor.matmul(mp[:, :H2], ones_row, b2row, start=False, stop=True,
                             skip_group_check=True)
            nc.scalar.activation(out=c2, in_=mp[:, :H2], func=AF.Relu)

            c2T = psmall.tile([H2, P], F32, tag="c2T")
            tp = ppa.tile([P, P], F32, tag="ppa")
            nc.tensor.transpose(tp[:H2, :P], c2, ident)
            nc.scalar.copy(out=c2T, in_=tp[:H2, :P])

            c_sb = psmall.tile([P, HK], F32, tag="c_sb")
            mp = ppa.tile([P, P], F32, tag="ppa")
            nc.tensor.matmul(mp[:, :HK], c2T, W3T, start=True, stop=False,
                             skip_group_check=True)
            nc.tensor.matmul(mp[:, :HK], ones_row, b3row, start=False, stop=True,
                             skip_group_check=True)
            nc.scalar.activation(out=c_sb, in_=mp[:, :HK], func=AF.Relu)

            # 1/|cent_hk| folded into c:  cs[:, hk] = c[:, hk] / cn[hk]
            csq = psmall.tile([P, HK], F32, tag="csq")
            nc.scalar.activation(out=csq, in_=c_sb, func=AF.Square)
            cn2p = ppa.tile([P, P], F32, tag="ppa")
            nc.tensor.matmul(cn2p[:1, :HK], ones_col, csq,
                             start=True, stop=True, skip_group_check=True)
            cn2s = psmall.tile([1, HK], F32, tag="cn2s")
            nc.scalar.copy(out=cn2s, in_=cn2p[:1, :HK])
            rcn = psmall.tile([1, HK], F32, tag="rcn")
            ra = psmall.tile([1, HK], F32, tag="ra")
            rb = psmall.tile([1, HK], F32, tag="rb")
            nr_sqrt_recip(rcn, cn2s, ra, rb)
            bcp = ppa.tile([P, P], F32, tag="ppa")
            nc.tensor.matmul(bcp[:, :HK], ones_row, rcn, start=True, stop=True,
                             skip_group_check=True)
            cs0 = psmall.tile([P, HK], F32, tag="cs0")
            nc.vector.tensor_mul(cs0, c_sb, bcp[:, :HK])

            # 16 row-masked copies of cs: cz[j] keeps d-rows [8j, 8j+8)
            cz = []
            for j in range(16):
                czj = psmall.tile([P, HK], F32, tag=f"cz{j}", name=f"cz{j}")
                nc.vector.tensor_scalar_mul(
                    out=czj, in0=cs0, scalar1=mask16[:, j : j + 1]
                )
                cz.append(czj)

            # --- cosine sims S[n, hk] ---
            # Two 64-row halves (partition bases 0/64) run concurrently on
            # the PE; each accumulates 8 masked contract-64 matmuls in PSUM.
            # Zero rows add exact zeros, so each matmul is numerically a
            # seq-8 partial dot and the PSUM chain is the f32 combine.
            s_all = ps_all.tile([P, NT, H, K], F32, tag="s_all")
            if LAST < P:
                for p0 in range(LAST, P, 32):
                    nc.vector.memset(s_all[p0 : min(p0 + 32, P), NT - 1], 0.0)
            dts = psmall.tile([P, HK], F32, tag="dts")
            for t in range(NT):
                pt = _tile_rows(t, n_nodes)
                dp0 = ppd.tile([P, HK], F32, tag="ppd0")
                dp1 = ppd.tile([P, HK], F32, tag="ppd1")
                for j in range(8):
                    for hf, dp in ((0, dp0), (1, dp1)):
                        base = 64 * hf
                        nc.tensor.matmul(
                            dp[:pt, :],
                            ndT[base : base + 64, t, :pt],
                            cz[8 * hf + j][base : base + 64, :],
                            start=(j == 0), stop=(j == 7),
                            skip_group_check=True,
                        )
                # TT ops may read at most one PSUM operand -> stage via SBUF
                nc.vector.tensor_copy(out=dts[:pt], in_=dp0[:pt, :])
                nc.vector.tensor_add(dts[:pt], dts[:pt], dp1[:pt, :])
                nc.vector.tensor_scalar_mul(
                    out=s_all[:pt, t].rearrange("p h k -> p (h k)"),
                    in0=dts[:pt],
                    scalar1=nn_r[:pt, t : t + 1],
                )

            norm = psmall.tile([P, NT, H], F32, tag="norm")
            nc.vector.reduce_sum(out=norm, in_=s_all, axis=mybir.AxisListType.X)
            nc.vector.tensor_scalar_add(out=norm, in0=norm, scalar1=1e-10)
            rnorm = psmall.tile([P, NT, H], F32, tag="rnorm")
            nc.vector.reciprocal(out=rnorm, in_=norm)

            ct = pct.tile([P, NT, K], F32, tag="ct")
            ct_tiles[b] = ct
            cn_tmp = psmall.tile([P, NT, K], F32, tag="cn_tmp")
            for h in range(H):
                rb_ = rnorm[:, :, h : h + 1].to_broadcast([P, NT, K])
                nc.vector.tensor_mul(cn_tmp, s_all[:, :, h, :], rb_)
                if h == 0:
                    nc.vector.tensor_scalar(
                        out=ct, in0=cn_tmp, scalar1=cwbc[:, 0:1],
                        scalar2=cbbc[:, 0:1], op0=ALU.mult, op1=ALU.add,
                    )
                else:
                    nc.vector.scalar_tensor_tensor(
                        out=ct, in0=cn_tmp, scalar=cwbc[:, h : h + 1], in1=ct,
                        op0=ALU.mult, op1=ALU.add,
                    )
            ct_r = pct.tile([P, NT, K], F32R, tag="ct_r")
            ctr_tiles[b] = ct_r
            nc.vector.tensor_copy(out=ct_r, in_=ct)

            # --- new_node = (C @ node) @ lin_w^T + lin_b ---
            vp = ppa.tile([P, P], F32, tag="ppa")
            for t in range(NT):
                pt = _tile_rows(t, n_nodes)
                nc.tensor.matmul(vp[:K, :D], ct[:pt, t, :], node_sb[:pt, t, :],
                                 start=(t == 0), stop=(t == NT - 1),
                                 skip_group_check=True)
            v_sb = psmall.tile([K, D], F32, tag="v_sb")
            nc.scalar.copy(out=v_sb, in_=vp[:K, :D])
            tp = ppa.tile([P, P], F32, tag="ppa")
            nc.tensor.transpose(tp[:D, :K], v_sb, ident[:K, :K])
            vT = psmall.tile([D, K], F32, tag="vT")
            nc.scalar.copy(out=vT, in_=tp[:D, :K])

            np_ = ppa.tile([P, P], F32, tag="ppa")
            nc.tensor.matmul(np_[:K, :DOUT], vT, lwT, start=True, stop=False,
                             skip_group_check=True)
            nc.tensor.matmul(np_[:K, :DOUT], ones_row[:, :K], lbrow,
                             start=False, stop=True, skip_group_check=True)
            stage = psmall.tile([K, OUTW], F32, tag="stage")
            stage_tiles[b] = stage
            nc.scalar.copy(out=stage[:, :DOUT], in_=np_[:K, :DOUT])

        def big_phase(b):
            ct = ct_tiles[b]
            ct_r = ctr_tiles[b]
            stage = stage_tiles[b]
            na_acc = psmall.tile([K, K], F32, tag="na_acc")
            nc.vector.memset(na_acc, 0.0)

            m0 = 0
            while m0 < n_nodes:
                mw = min(MCH, n_nodes - m0)
                tcp = ppc.tile([K, MCH], F32, tag="tc")
                for t in range(NT):
                    pt = _tile_rows(t, n_nodes)
                    at = padj.tile([P, MCH], F32R, tag="adj")
                    nc.sync.dma_start(
                        out=at[:pt, :mw],
                        in_=adj_e[b, t * P : t * P + pt, m0 : m0 + mw],
                    )
                    nc.tensor.matmul(
                        tcp[:, :mw],
                        ct_r[:pt, t, :],
                        at[:pt, :mw],
                        start=(t == 0), stop=(t == NT - 1),
                        skip_group_check=True,
                    )
                t_sb = ptsb.tile([K, MCH], F32, tag="tsb")
                nc.vector.tensor_copy(out=t_sb[:, :mw], in_=tcp[:, :mw])

                nj = int(math.ceil(mw / P))
                nap = ppn.tile([K, K], F32, tag="na")
                for j in range(nj):
                    jw = min(P, mw - j * P)
                    tm = (m0 + j * P) // P
                    ttp = ppt.tile([P, K], F32, tag="tt")
                    nc.tensor.transpose(
                        ttp[:jw, :K], t_sb[:K, j * P : j * P + jw], ident[:K, :K]
                    )
                    tt_sb = psmall.tile([P, K], F32, tag="tt_sb")
                    nc.scalar.copy(out=tt_sb[:jw, :], in_=ttp[:jw, :K])
                    nc.tensor.matmul(
                        nap, tt_sb[:jw, :], ct[:jw, tm, :],
                        start=(j == 0), stop=(j == nj - 1),
                        skip_group_check=True,
                    )
                nc.vector.tensor_add(na_acc, na_acc, nap)
                m0 += mw

            nc.scalar.activation(out=stage[:, DOUT:], in_=na_acc, func=AF.Relu)
            nc.sync.dma_start(out=out_e[b], in_=stage)

        for b in range(bpc):
            sim_phase(b)
        for b in range(bpc):
            big_phase(b)

    return nc


_COMPILED = {}


def _get_program(n_nodes=N, bpc=BPC):
    key = (n_nodes, bpc)
    if key not in _COMPILED:
        nc = bacc.Bacc("TRN2", target_bir_lowering=False)
        build_program(nc, n_nodes=n_nodes, bpc=bpc)
        nc.compile()
        _COMPILED[key] = nc
    return _COMPILED[key]


LAST_RESULTS = None


def kernel(node_set, adj, w1, b1, W2, b2, W3, b3, conv_w, conv_b, lin_w, lin_b,
           trace=False):
    global LAST_RESULTS
    node_set = np.ascontiguousarray(np.asarray(node_set, dtype=np.float32))
    adj = np.ascontiguousarray(np.asarray(adj, dtype=np.float32))
    small = {
        "w1": w1, "b1": b1, "W2": W2, "b2": b2, "W3": W3, "b3": b3,
        "conv_w": conv_w, "conv_b": conv_b, "lin_w": lin_w, "lin_b": lin_b,
    }
    small = {k: np.ascontiguousarray(np.asarray(v, dtype=np.float32))
             for k, v in small.items()}

    nc = _get_program()
    in_maps = []
    for i in range(NCORES):
        m = {"nodes": node_set[i * BPC : (i + 1) * BPC],
             "adj": adj[i * BPC : (i + 1) * BPC]}
        m.update(small)
        in_maps.append(m)

    res = run_bass_kernel_spmd(nc, in_maps, core_ids=list(range(NCORES)),
                               trace=trace)
    LAST_RESULTS = res
    out = np.stack([res.results[i]["out"] for i in range(NCORES)], axis=0)
    out = out.reshape(B, K, OUTW)
    new_node_set = np.ascontiguousarray(out[:, :, :DOUT])
    new_adj = np.ascontiguousarray(out[:, :, DOUT:])
    return new_node_set, new_adj
